# revision 2
# baseline (speedup 1.0000x reference)
"""Trainium2 Bass kernel for nn_ConditionalFeaturesUpsample.

Reference computation (B=1, L=64, C=80):
    x   = local_features[0].T                          # [80, 64]
    up  = ConvTranspose1d(x; wt, bt, k=stride=4)       # [80, 256]
    y   = w1 @ up + b1                                 # [3072, 256]
    out = tile(y, 75) reshaped to [128, 1, 24, 19200]  # out[ch,0,l,t] = y[l*128+ch, t%256]

Sharding: tensor-parallel over the 3072 output channels (batch is 1).
Core i computes channel rows {l*128 + 16*i + j}, i.e. the slice
out[16*i:16*(i+1), 0, :, :]; the host gather is a concat + transpose.

Host-side weight preprocessing (pure algebra, no activations touched):
    W2[m,c,k] = sum_o w1[m,o] * wt[c,o,k]   (ConvT folded into the 1x1 conv)
    b_eff     = w1 @ bt + b1
so each core runs 12 matmuls [80 -> 128, 64] straight from x, then adds
b_eff while rearranging PSUM [m,(k,l)] -> SBUF [m, 4l+k].

The kernel is HBM-write-bound: the per-core output shard is 24*16*19200
elements. Everything (weights, x, output) is stored fp16 — halving HBM
traffic vs f32 — while matmuls accumulate in f32 PSUM; measured rel err
~1e-3 against the f32 reference vs the 2e-2 gate. The 75x time-repeat is
never materialized in SBUF: a [128, CHUNK] tile (CHUNK/256 periods) is
built per group and broadcast-source DMAs (zero-stride repeat AP) write
the whole 19200-wide span. Weights ship in two packed fp16 tensors so
group 0's matmuls start as soon as the first (smaller) DMA lands.
"""
import os
import sys

import numpy as np

for _p in ("/opt/trn_rl_repo", "/root/.axon_site/_ro/trn_rl_repo"):
    if os.path.isdir(_p) and _p not in sys.path:
        sys.path.append(_p)

import concourse.bacc as bacc
import concourse.mybir as mybir
import concourse.tile as tile
from concourse.bass_utils import run_bass_kernel_spmd

UPSAMPLE_REPEAT = 75
NUM_LAYERS = 24
N_CORES = 8
GROUPS = 3             # groups of 128 channel-rows per core
T_SMALL = 256
T_FULL = T_SMALL * UPSAMPLE_REPEAT  # 19200
F16 = mybir.dt.float16

CHUNK = 1536           # periods per broadcast-source tile (multiple of 256)
N_REP = T_FULL // CHUNK
TAIL = T_FULL - N_REP * CHUNK  # remainder, sourced from y_mid[:, :TAIL]

# par1 [128, 579]: [0:3) b_eff | [3:67) x | [67:579) W2 g0 (4 lhsT chunks of 128)
# par2 [128, 1024]: W2 g1, g2 (8 lhsT chunks of 128)
P1_BE, P1_X, P1_W2, P1_COLS = 0, 3, 67, 579
P2_COLS = 1024


def build_bass():
    nc = bacc.Bacc()
    par1_d = nc.declare_dram_parameter("par1", [128, P1_COLS], F16, isOutput=False)
    par2_d = nc.declare_dram_parameter("par2", [128, P2_COLS], F16, isOutput=False)
    # l-major per-core output: out[l, j, t] = y[(8g+l)*128 + 16*core + j, t%256]
    out_d = nc.declare_dram_parameter("out", [NUM_LAYERS, 16, T_FULL], F16, isOutput=True)

    with tile.TileContext(nc) as tc:
        with (
            tc.tile_pool(name="consts", bufs=1) as consts,
            tc.tile_pool(name="psum", bufs=2, space="PSUM") as psum_pool,
            tc.tile_pool(name="mid", bufs=3) as mid_pool,
        ):
            par1_sb = consts.tile([128, P1_COLS], F16)
            nc.sync.dma_start(out=par1_sb[:], in_=par1_d[:])
            par2_sb = consts.tile([128, P2_COLS], F16)
            nc.sync.dma_start(out=par2_sb[:], in_=par2_d[:])
            be_sb = par1_sb[:, P1_BE:P1_X]
            x_sb = par1_sb[0:80, P1_X:P1_W2]

            def w2chunk(g, k):
                if g == 0:
                    return par1_sb[0:80, P1_W2 + 128 * k:P1_W2 + 128 * (k + 1)]
                off = 128 * (4 * (g - 1) + k)
                return par2_sb[0:80, off:off + 128]

            for g in range(GROUPS):
                y_ps = psum_pool.tile([128, T_SMALL], mybir.dt.float32, tag="y_ps")
                for k in range(4):
                    nc.tensor.matmul(
                        y_ps[:, 64 * k:64 * (k + 1)],
                        lhsT=w2chunk(g, k),
                        rhs=x_sb,
                        start=True,
                        stop=True,
                    )
                y_mid = mid_pool.tile([128, CHUNK], F16, tag="y_mid")
                # PSUM [m,(k,l)] -> SBUF [m, 4l+k] with per-partition bias add
                nc.scalar.activation(
                    out=y_mid[:, :T_SMALL].rearrange("p (l k) -> p k l", k=4),
                    in_=y_ps[:].rearrange("p (k l) -> p k l", k=4),
                    func=mybir.ActivationFunctionType.Identity,
                    bias=be_sb[:, g:g + 1],
                )
                # Fill the remaining periods by doubling
                filled = T_SMALL
                while filled < CHUNK:
                    n = min(filled, CHUNK - filled)
                    nc.vector.tensor_copy(
                        out=y_mid[:, filled:filled + n], in_=y_mid[:, :n]
                    )
                    filled += n
                # Broadcast-source DMAs write all 75 periods; group rows
                # (l,j) are contiguous in the l-major layout.
                grp = out_d[8 * g:8 * (g + 1), :, :].rearrange("l j t -> (l j) t")
                nc.sync.dma_start(
                    out=grp[:, :N_REP * CHUNK],
                    in_=y_mid[:].unsqueeze(1).broadcast_to([128, N_REP, CHUNK]),
                )
                if TAIL:
                    nc.sync.dma_start(
                        out=grp[:, N_REP * CHUNK:],
                        in_=y_mid[:, :TAIL],
                    )
    nc.compile()
    return nc


def host_prep(local_features, wt, bt, w1, b1):
    lf = np.asarray(local_features, np.float32)
    wt64 = np.asarray(wt, np.float64)
    w164 = np.asarray(w1, np.float64)
    x = lf[0].T.astype(np.float16)                           # [80, 64]
    W2 = np.einsum('mo,cok->mck', w164, wt64).astype(np.float16)  # [3072,80,4]
    b_eff = (w164 @ np.asarray(bt, np.float64)
             + np.asarray(b1, np.float64)).astype(np.float16)

    # Channel row for (core, g, p): c = (8g + p//16)*128 + 16*core + p%16
    g_idx = np.arange(GROUPS)[:, None]
    p_idx = np.arange(128)[None, :]
    base = (8 * g_idx + p_idx // 16) * 128 + p_idx % 16      # l-major partitions
    in_maps = []
    for core in range(N_CORES):
        c = base + 16 * core                                 # [3, 128]
        W2sel = W2[c]                                        # [3, 128, 80, 4]
        par1 = np.zeros((128, P1_COLS), np.float16)
        par1[:, P1_BE:P1_X] = b_eff[c].T
        par1[0:80, P1_X:P1_W2] = x
        par1[0:80, P1_W2:] = np.concatenate(
            [W2sel[0, :, :, k].T for k in range(4)], axis=1)
        par2 = np.zeros((128, P2_COLS), np.float16)
        par2[0:80, :] = np.concatenate(
            [W2sel[g, :, :, k].T for g in (1, 2) for k in range(4)], axis=1)
        in_maps.append({"par1": par1, "par2": par2})
    return in_maps


def run(inputs, trace=False, **spmd_kwargs):
    """Returns (full_output [128,1,24,19200], BassKernelResults)."""
    nc = build_bass()
    in_maps = host_prep(**inputs)
    res = run_bass_kernel_spmd(
        nc, in_maps, core_ids=list(range(N_CORES)), trace=trace, **spmd_kwargs
    )
    out = np.empty((128, 1, NUM_LAYERS, T_FULL), np.float32)
    for i in range(N_CORES):
        shard = np.asarray(res.results[i]["out"])    # [24, 16, 19200] fp16
        out[16 * i:16 * (i + 1), 0] = shard.transpose(1, 0, 2)
    return out, res


def kernel(**inputs):
    out, _ = run(inputs, trace=False)
    return out
